# revision 1
# baseline (speedup 1.0000x reference)
"""Trainium2 Bass kernel for a 2-layer GRU decoder with FC head + softmax feedback.

Model (per time step, T=64 steps, strictly sequential):
    h0 = GRUCell0(out, h0)   # input D=256 -> H=1024
    h1 = GRUCell1(h0, h1)    # H -> H
    out = softmax(fc3(gelu(LN2(fc2(gelu(LN1(fc1(h1))))))))

Sharding: pure data-parallel over batch (256 -> 32 per core, 8 cores),
weights replicated, zero collectives.

Layout: feature-major. An activation of F features for the 32 local batch
rows lives in SBUF as [128 partitions, (F/128)*32], column = chunk*32 + b.
Matmuls use weights as the stationary operand (lhsT = W.T chunk [128,128]
bf16) and activations as the moving operand ([128, 32] bf16), f32 PSUM.

Scheduling: the W_hh-side matmuls of step t+1 depend only on h(t), so they
are emitted at the end of step t's body — the Tile scheduler runs them in
the PE gaps while step t's LN/softmax vector chains unwind.

ACT engine uses a single activation table (sigmoid_and_others = {sigmoid,
tanh, erf, copy, square}): gelu is computed via Erf, softmax's exp via
sigma(x)/sigma(-x), and LN's rsqrt via a DVE Newton iteration — an
activation-table switch costs ~1.3us on this hardware.

Biases are injected into PSUM with one block-diagonal matmul per region
(lhsT = bias rows [C,128], rhs = block-diagonal ones [C, C*32]) because the
vector engines here cannot use broadcast (3D) access patterns.  LN / softmax
cross-partition sums use an all-ones [128,128] stationary so the sums arrive
replicated across all partitions.
"""

import os
import json
import numpy as np
import ml_dtypes
from contextlib import ExitStack

import concourse.bass as bass
import concourse.tile as tile
from concourse import mybir
from concourse.bass_utils import run_bass_kernel_spmd

BF16 = ml_dtypes.bfloat16
f32, bf16, i32 = mybir.dt.float32, mybir.dt.bfloat16, mybir.dt.int32
FT, ALU = mybir.ActivationFunctionType, mybir.AluOpType

B, T, D, H = 256, 64, 256, 1024
FC1, FC2 = 1024, 512
EPS = 1e-5
NCORES = 8
BL = B // NCORES            # 32 batch rows per core
P = 128
KD, KH, KF2 = D // P, H // P, FC2 // P    # 2, 8, 4
M1, M2, M3 = FC1 // P, FC2 // P, D // P   # 8, 4, 2
MRZ, MN = 2 * KH, KH                      # 16 rz chunks, 8 n chunks

_cache = {}
last_result = None
_USE_LNB = [True]   # apply LN beta adds (skipped when the inputs' ln betas are 0)


def _split_multiwait_json(raw: bytes) -> bytes:
    """The walrus build here encodes at most one sem-wait per instruction;
    hoist extra waits onto standalone NoOps inserted just before, on the
    same engine (wait order is preserved by the engine's sequencer)."""
    j = json.loads(raw)
    ctr = 0
    for fn in j.get("functions", []):
        for bb in fn.get("blocks", []):
            out = []
            for inst in bb.get("instructions", []):
                si = inst.get("sync_info")
                waits = (si.get("on_wait") or []) if si else []
                if len(waits) > 1:
                    for w in waits[:-1]:
                        ctr += 1
                        out.append({
                            "debug": inst.get("debug", 0),
                            "engine": inst["engine"],
                            "ins": [], "outs": [],
                            "name": f"swx-{ctr}",
                            "opcode": "NoOp",
                            "sync_info": {"on_wait": [w], "on_update": []},
                        })
                    si["on_wait"] = [waits[-1]]
                out.append(inst)
            bb["instructions"] = out
    return json.dumps(j).encode()


def _patch_serialization(nc):
    orig = nc.to_json_bytes
    nc.to_json_bytes = lambda: _split_multiwait_json(orig())
    return nc


def _build(t_steps=T, repeat=1):
    nc = bass.Bass()

    # ---- DRAM parameters -------------------------------------------------
    wih0 = nc.declare_dram_parameter("wih0", [P, KD * 3 * H], bf16, isOutput=False)
    whh0 = nc.declare_dram_parameter("whh0", [P, KH * 3 * H], bf16, isOutput=False)
    wih1 = nc.declare_dram_parameter("wih1", [P, KH * 3 * H], bf16, isOutput=False)
    whh1 = nc.declare_dram_parameter("whh1", [P, KH * 3 * H], bf16, isOutput=False)
    wf1 = nc.declare_dram_parameter("wf1", [P, KH * FC1], bf16, isOutput=False)
    wf2 = nc.declare_dram_parameter("wf2", [P, M1 * FC2], bf16, isOutput=False)
    wf3 = nc.declare_dram_parameter("wf3", [P, KF2 * D], bf16, isOutput=False)

    # bias rows for block-diag injection: [C, 128] with row c = feature chunk c
    tbrz0 = nc.declare_dram_parameter("tbrz0", [MRZ, P], bf16, isOutput=False)
    tbn0 = nc.declare_dram_parameter("tbn0", [2 * MN, P], bf16, isOutput=False)
    tbrz1 = nc.declare_dram_parameter("tbrz1", [MRZ, P], bf16, isOutput=False)
    tbn1 = nc.declare_dram_parameter("tbn1", [2 * MN, P], bf16, isOutput=False)
    tbf1 = nc.declare_dram_parameter("tbf1", [M1, P], bf16, isOutput=False)
    tbf23 = nc.declare_dram_parameter("tbf23", [M2 + M3, P], bf16, isOutput=False)
    bdg = nc.declare_dram_parameter("bdg", [MRZ, MRZ * BL], bf16, isOutput=False)

    g1d = nc.declare_dram_parameter("g1", [P, M1], f32, isOutput=False)
    gb1d = nc.declare_dram_parameter("gb1", [P, M1], f32, isOutput=False)
    g2d = nc.declare_dram_parameter("g2", [P, M2], f32, isOutput=False)
    gb2d = nc.declare_dram_parameter("gb2", [P, M2], f32, isOutput=False)

    h0d = nc.declare_dram_parameter("h0f", [P, KH * BL], f32, isOutput=False)
    h0bd = nc.declare_dram_parameter("h0b", [P, KH * BL], bf16, isOutput=False)
    h1d = nc.declare_dram_parameter("h1f", [P, KH * BL], f32, isOutput=False)
    h1bd = nc.declare_dram_parameter("h1b", [P, KH * BL], bf16, isOutput=False)

    outd = nc.declare_dram_parameter("out", [t_steps, P, KD * BL], f32, isOutput=True)

    with ExitStack() as ctx:
        tc = ctx.enter_context(tile.TileContext(nc))
        wp = ctx.enter_context(tc.tile_pool(name="wp", bufs=1))
        st = ctx.enter_context(tc.tile_pool(name="st", bufs=2))
        tp = ctx.enter_context(tc.tile_pool(name="tp", bufs=2))
        pg = ctx.enter_context(tc.tile_pool(name="pg", bufs=2, space="PSUM"))
        pf = ctx.enter_context(tc.tile_pool(name="pf", bufs=1, space="PSUM"))
        pstat = ctx.enter_context(tc.tile_pool(name="pstat", bufs=2, space="PSUM"))

        # ---- load weights/biases into SBUF (resident) --------------------
        def load(dram, dtype):
            tl = wp.tile(dram.shape, dtype, tag=dram.name)
            nc.sync.dma_start(out=tl[:], in_=dram[:])
            return tl

        Wih0, Whh0 = load(wih0, bf16), load(whh0, bf16)
        Wih1, Whh1 = load(wih1, bf16), load(whh1, bf16)
        Wf1, Wf2, Wf3 = load(wf1, bf16), load(wf2, bf16), load(wf3, bf16)
        Tbrz0, Tbn0 = load(tbrz0, bf16), load(tbn0, bf16)
        Tbrz1, Tbn1 = load(tbrz1, bf16), load(tbn1, bf16)
        Tbf1, Tbf23 = load(tbf1, bf16), load(tbf23, bf16)
        Bd = load(bdg, bf16)
        G1, Gb1, G2, Gb2 = load(g1d, f32), load(gb1d, f32), load(g2d, f32), load(gb2d, f32)

        ones_sq = wp.tile([P, P], f32)   # all-ones stationary: colsum bcast to all parts
        nc.vector.memset(ones_sq[:], 1.0)

        # ---- state tiles --------------------------------------------------
        h0 = st.tile([P, KH * BL], f32, tag="h0")
        h0b = st.tile([P, KH * BL], bf16, tag="h0b")
        h1 = st.tile([P, KH * BL], f32, tag="h1")
        h1b = st.tile([P, KH * BL], bf16, tag="h1b")
        ob = st.tile([P, KD * BL], bf16, tag="ob")
        nc.sync.dma_start(out=h0[:], in_=h0d[:])
        nc.sync.dma_start(out=h0b[:], in_=h0bd[:])
        nc.sync.dma_start(out=h1[:], in_=h1d[:])
        nc.sync.dma_start(out=h1b[:], in_=h1bd[:])
        nc.vector.memset(ob[:], 0.0)

        def mm(out_ap, w_tile, k, m, rhs, first, last, n_out=3 * H):
            nc.tensor.matmul(
                out_ap,
                lhsT=w_tile[:, k * n_out + m * P:k * n_out + (m + 1) * P],
                rhs=rhs[:, k * BL:(k + 1) * BL],
                start=first, stop=last,
                skip_group_check=True,
            )

        def bias_mm(region_ap, biasT, nrows):
            nc.tensor.matmul(
                region_ap,
                lhsT=biasT[0:nrows, :],
                rhs=Bd[0:nrows, 0:nrows * BL],
                start=True, stop=False,
                skip_group_check=True,
            )

        def gru_prefetch(Whh, hb, TbrzL, TbnL):
            """Bias injection + all W_hh@h matmuls for the NEXT GRU step.
            Depends only on h (ready), so it fills PE gaps under vector work."""
            ps = pg.tile([P, (MRZ + 2 * MN) * BL], f32, tag="gru")
            rz = ps[:, 0:MRZ * BL]
            hnn = ps[:, (MRZ + MN) * BL:(MRZ + 2 * MN) * BL]
            inhn = ps[:, MRZ * BL:(MRZ + 2 * MN) * BL]
            bias_mm(rz, TbrzL, MRZ)
            bias_mm(inhn, TbnL, 2 * MN)
            for m in range(MRZ):
                o = rz[:, m * BL:(m + 1) * BL]
                for k in range(KH):
                    mm(o, Whh, k, m, hb, False, False)
            for m in range(MN):
                o = hnn[:, m * BL:(m + 1) * BL]
                for k in range(KH):
                    mm(o, Whh, k, MRZ + m, hb, False, k == KH - 1)
            return ps

        def gru_finish(ps, xb, kx, Wih, hf, tag):
            """W_ih@x matmuls + gate math; returns (h' f32, h' bf16)."""
            rz = ps[:, 0:MRZ * BL]
            inn = ps[:, MRZ * BL:(MRZ + MN) * BL]
            hnn = ps[:, (MRZ + MN) * BL:(MRZ + 2 * MN) * BL]
            for m in range(MRZ):
                o = rz[:, m * BL:(m + 1) * BL]
                for k in range(kx):
                    mm(o, Wih, k, m, xb, False, k == kx - 1)
            for m in range(MN):
                o = inn[:, m * BL:(m + 1) * BL]
                for k in range(kx):
                    mm(o, Wih, k, MRZ + m, xb, False, k == kx - 1)

            rzs = tp.tile([P, MRZ * BL], f32, tag="rzs")
            nc.scalar.activation(out=rzs[:], in_=rz, func=FT.Sigmoid)
            a1 = tp.tile([P, MN * BL], f32, tag="a1")
            nc.vector.tensor_tensor(out=a1[:], in0=rzs[:, 0:MN * BL], in1=hnn,
                                    op=ALU.mult)
            nc.vector.tensor_tensor(out=a1[:], in0=a1[:], in1=inn, op=ALU.add)
            n_t = tp.tile([P, MN * BL], f32, tag="a1")
            nc.scalar.activation(out=n_t[:], in_=a1[:], func=FT.Tanh)
            # h' = n + z*(h - n)
            d = tp.tile([P, MN * BL], f32, tag="big")
            nc.vector.tensor_tensor(out=d[:], in0=hf[:], in1=n_t[:], op=ALU.subtract)
            nc.vector.tensor_tensor(out=d[:], in0=d[:], in1=rzs[:, MN * BL:MRZ * BL],
                                    op=ALU.mult)
            hn_f = st.tile([P, KH * BL], f32, tag=tag)
            nc.vector.tensor_tensor(out=hn_f[:], in0=n_t[:], in1=d[:], op=ALU.add)
            hn_b = st.tile([P, KH * BL], bf16, tag=tag + "b")
            nc.vector.tensor_copy(out=hn_b[:], in_=hn_f[:])
            return hn_f, hn_b

        def rsqrt_dve(v):
            """rstd = 1/sqrt(v) on DVE: Quake seed + 2 Newton iterations.
            Avoids the sqrt activation table (~1.3us table switch)."""
            vi = tp.tile([P, BL], i32, tag="vi", bufs=1)
            nc.vector.tensor_scalar(out=vi[:], in0=v[:].bitcast(i32),
                                    scalar1=1, scalar2=None,
                                    op0=ALU.arith_shift_right)
            nc.vector.tensor_scalar(out=vi[:], in0=vi[:],
                                    scalar1=-1, scalar2=0x5F3759DF,
                                    op0=ALU.mult, op1=ALU.add)
            y0 = vi[:].bitcast(f32)
            y = tp.tile([P, BL], f32, tag="rstd", bufs=1)
            r = tp.tile([P, BL], f32, tag="nwt", bufs=1)
            nc.vector.tensor_tensor(out=r[:], in0=y0, in1=y0, op=ALU.mult)
            nc.vector.tensor_tensor(out=r[:], in0=r[:], in1=v[:], op=ALU.mult)
            nc.vector.tensor_scalar(out=r[:], in0=r[:], scalar1=-0.5, scalar2=1.5,
                                    op0=ALU.mult, op1=ALU.add)
            nc.vector.tensor_tensor(out=y[:], in0=y0, in1=r[:], op=ALU.mult)
            nc.vector.tensor_tensor(out=r[:], in0=y[:], in1=y[:], op=ALU.mult)
            nc.vector.tensor_tensor(out=r[:], in0=r[:], in1=v[:], op=ALU.mult)
            nc.vector.tensor_scalar(out=r[:], in0=r[:], scalar1=-0.5, scalar2=1.5,
                                    op0=ALU.mult, op1=ALU.add)
            nc.vector.tensor_tensor(out=y[:], in0=y[:], in1=r[:], op=ALU.mult)
            return y

        def ln_gelu(y, nchunk, s_ps, feat, G, Gb, out_tag):
            """In-place LN on y, then 2*gelu via erf (the 0.5 is folded into
            the next layer's weights on the host). Returns bf16 tile."""
            sq = tp.tile([P, nchunk * BL], f32, tag="big")
            nc.scalar.activation(out=sq[:], in_=y[:], func=FT.Square)
            s1 = s_ps[:, 0:BL]
            s2 = s_ps[:, BL:2 * BL]
            for k in range(nchunk):
                nc.tensor.matmul(s1, lhsT=ones_sq[:], rhs=y[:, k * BL:(k + 1) * BL],
                                 start=k == 0, stop=k == nchunk - 1)
            for k in range(nchunk):
                nc.tensor.matmul(s2, lhsT=ones_sq[:], rhs=sq[:, k * BL:(k + 1) * BL],
                                 start=k == 0, stop=k == nchunk - 1)
            mu = tp.tile([P, BL], f32, tag="mu", bufs=1)
            q = tp.tile([P, BL], f32, tag="qf", bufs=1)
            nc.scalar.activation(out=mu[:], in_=s1, func=FT.Copy, scale=1.0 / feat)
            nc.scalar.activation(out=q[:], in_=s2, func=FT.Copy, scale=1.0 / feat)
            mu2 = tp.tile([P, BL], f32, tag="mu2", bufs=1)
            nc.scalar.activation(out=mu2[:], in_=mu[:], func=FT.Square)
            nc.vector.tensor_tensor(out=q[:], in0=q[:], in1=mu2[:], op=ALU.subtract)
            nc.vector.tensor_scalar_add(out=q[:], in0=q[:], scalar1=EPS)
            rstd = rsqrt_dve(q)
            vc = tp.tile([P, BL], f32, tag="vc", bufs=1)
            for c in range(nchunk):
                yc = y[:, c * BL:(c + 1) * BL]
                nc.vector.tensor_tensor(out=vc[:], in0=yc, in1=mu[:], op=ALU.subtract)
                nc.vector.scalar_tensor_tensor(
                    out=yc, in0=vc[:], scalar=G[:, c:c + 1], in1=rstd[:],
                    op0=ALU.mult, op1=ALU.mult)
                if _USE_LNB[0]:
                    nc.vector.tensor_scalar_add(out=yc, in0=yc, scalar1=Gb[:, c:c + 1])
            e = tp.tile([P, nchunk * BL], f32, tag="big")
            nc.scalar.activation(out=e[:], in_=y[:], func=FT.Erf, scale=0.7071067811865476)
            gb_t = tp.tile([P, nchunk * BL], bf16, tag=out_tag)
            nc.vector.scalar_tensor_tensor(out=gb_t[:], in0=e[:], scalar=1.0,
                                           in1=y[:], op0=ALU.add, op1=ALU.mult)
            return gb_t

        # ---- time loop ----------------------------------------------------
        ps0 = gru_prefetch(Whh0, h0b, Tbrz0, Tbn0)
        ps1 = gru_prefetch(Whh1, h1b, Tbrz1, Tbn1)
        xb, kx = ob, KD
        for t in range(t_steps * repeat):
            t_out = t % t_steps
            h0, h0b = gru_finish(ps0, xb, kx, Wih0, h0, "h0")
            h1, h1b = gru_finish(ps1, h0b, KH, Wih1, h1, "h1")

            # ---- fc1 ----
            f1 = pf.tile([P, M1 * BL], f32, tag="f1")
            bias_mm(f1[:], Tbf1, M1)
            for m in range(M1):
                o = f1[:, m * BL:(m + 1) * BL]
                for k in range(KH):
                    mm(o, Wf1, k, m, h1b, False, k == KH - 1, n_out=FC1)
            stat = pstat.tile([P, 5 * BL], f32, tag="stat")
            y1 = tp.tile([P, M1 * BL], f32, tag="y1")
            nc.scalar.activation(out=y1[:], in_=f1[:], func=FT.Copy)
            g1b = ln_gelu(y1, M1, stat[:, 0:2 * BL], FC1, G1, Gb1, "g1b")

            # ---- fc2 + LN2 + gelu ----
            hd = pf.tile([P, (M2 + M3) * BL], f32, tag="hd")
            f2 = hd[:, 0:M2 * BL]
            bias_mm(hd[:], Tbf23, M2 + M3)
            for m in range(M2):
                o = f2[:, m * BL:(m + 1) * BL]
                for k in range(M1):
                    mm(o, Wf2, k, m, g1b, False, k == M1 - 1, n_out=FC2)
            y2 = tp.tile([P, M2 * BL], f32, tag="y2")
            nc.scalar.activation(out=y2[:], in_=f2, func=FT.Copy)
            g2b = ln_gelu(y2, M2, stat[:, 2 * BL:4 * BL], FC2, G2, Gb2, "g2b")

            # ---- fc3 + softmax (exp via sigma(x)/sigma(-x)) ----
            f3 = hd[:, M2 * BL:(M2 + M3) * BL]
            for m in range(M3):
                o = f3[:, m * BL:(m + 1) * BL]
                for k in range(KF2):
                    mm(o, Wf3, k, m, g2b, False, k == KF2 - 1, n_out=D)
            sp = tp.tile([P, M3 * BL], f32, tag="es")
            nc.scalar.activation(out=sp[:], in_=f3, func=FT.Sigmoid)
            sn = tp.tile([P, M3 * BL], f32, tag="es2")
            nc.scalar.activation(out=sn[:], in_=f3, func=FT.Sigmoid, scale=-1.0)
            nc.vector.reciprocal(out=sn[:], in_=sn[:])
            nc.vector.tensor_tensor(out=sp[:], in0=sp[:], in1=sn[:], op=ALU.mult)
            ssum = stat[:, 4 * BL:5 * BL]
            for k in range(M3):
                nc.tensor.matmul(ssum, lhsT=ones_sq[:], rhs=sp[:, k * BL:(k + 1) * BL],
                                 start=k == 0, stop=k == M3 - 1)
            sinv = tp.tile([P, BL], f32, tag="sinv", bufs=1)
            nc.vector.tensor_copy(out=sinv[:], in_=ssum)
            nc.vector.reciprocal(out=sinv[:], in_=sinv[:])
            of = st.tile([P, KD * BL], f32, tag="of")
            for c in range(M3):
                nc.vector.tensor_tensor(out=of[:, c * BL:(c + 1) * BL],
                                        in0=sp[:, c * BL:(c + 1) * BL],
                                        in1=sinv[:], op=ALU.mult)
            ob = st.tile([P, KD * BL], bf16, tag="ob")
            nc.vector.tensor_copy(out=ob[:], in_=of[:])
            nc.sync.dma_start(out=outd[t_out], in_=of[:])

            # ---- prefetch next step's W_hh work (fills PE gaps above) ----
            if t < t_steps * repeat - 1:
                ps0 = gru_prefetch(Whh0, h0b, Tbrz0, Tbn0)
                ps1 = gru_prefetch(Whh1, h1b, Tbrz1, Tbn1)
            xb, kx = ob, KD

    return nc


def _prep_shared(inp):
    """Host-side weight/bias prep shared by all cores."""
    def wchunks(Wt):
        # Wt: [IN, OUT] = W.T ; -> [128, (IN/128)*OUT] bf16, free = k*OUT + out
        IN, OUT = Wt.shape
        k = IN // P
        return np.ascontiguousarray(
            Wt.reshape(k, P, OUT).transpose(1, 0, 2).reshape(P, k * OUT)
        ).astype(BF16)

    def rows(v):
        return np.ascontiguousarray(np.asarray(v).reshape(-1, P)).astype(BF16)

    def colmajor(v):
        return np.ascontiguousarray(np.asarray(v).reshape(-1, P).T).astype(np.float32)

    bd = np.zeros((MRZ, MRZ * BL), np.float32)
    for c in range(MRZ):
        bd[c, c * BL:(c + 1) * BL] = 1.0

    # gelu is computed as (1+erf(x/sqrt2))*x on device; fold the missing 0.5
    # into the consumer weights of g1b/g2b (fc2 and fc3).
    m = {
        "wih0": wchunks(np.asarray(inp["W_ih0"]).T),
        "whh0": wchunks(np.asarray(inp["W_hh0"]).T),
        "wih1": wchunks(np.asarray(inp["W_ih1"]).T),
        "whh1": wchunks(np.asarray(inp["W_hh1"]).T),
        "wf1": wchunks(np.asarray(inp["fc1_w"]).T),
        "wf2": wchunks(np.asarray(inp["fc2_w"]).T * 0.5),
        "wf3": wchunks(np.asarray(inp["fc3_w"]).T * 0.5),
        "tbrz0": rows(inp["b_ih0"][:2 * H] + inp["b_hh0"][:2 * H]),
        "tbn0": np.concatenate([rows(inp["b_ih0"][2 * H:]), rows(inp["b_hh0"][2 * H:])]),
        "tbrz1": rows(inp["b_ih1"][:2 * H] + inp["b_hh1"][:2 * H]),
        "tbn1": np.concatenate([rows(inp["b_ih1"][2 * H:]), rows(inp["b_hh1"][2 * H:])]),
        "tbf1": rows(inp["fc1_b"]),
        "tbf23": np.concatenate([rows(inp["fc2_b"]), rows(inp["fc3_b"])]),
        "bdg": bd.astype(BF16),
        "g1": colmajor(inp["ln1_g"]),
        "gb1": colmajor(inp["ln1_b"]),
        "g2": colmajor(inp["ln2_g"]),
        "gb2": colmajor(inp["ln2_b"]),
    }
    return m


def _feature_major(x):
    # x: [BL, F] f32 -> [128, (F/128)*BL], col = chunk*BL + b
    F = x.shape[1]
    k = F // P
    return np.ascontiguousarray(
        x.T.reshape(k, P, BL).transpose(1, 0, 2).reshape(P, k * BL)
    ).astype(np.float32)


def kernel(**inputs):
    global last_result
    inp = {k: np.asarray(v) for k, v in inputs.items()}
    t_steps = T
    use_lnb = bool(np.any(inp["ln1_b"]) or np.any(inp["ln2_b"]))
    key = (t_steps, use_lnb)
    if _cache.get("key") != key:
        _USE_LNB[0] = use_lnb
        _cache["nc"] = _patch_serialization(_build(t_steps))
        _cache["key"] = key
    nc = _cache["nc"]

    shared = _prep_shared(inp)
    in_maps = []
    for c in range(NCORES):
        sl = slice(c * BL, (c + 1) * BL)
        h0 = _feature_major(inp["hidden"][0, sl])
        h1 = _feature_major(inp["hidden"][1, sl])
        m = dict(shared)
        m["h0f"] = h0
        m["h0b"] = h0.astype(BF16)
        m["h1f"] = h1
        m["h1b"] = h1.astype(BF16)
        in_maps.append(m)

    trace = bool(int(os.environ.get("KERNEL_TRACE", "0")))
    res = run_bass_kernel_spmd(nc, in_maps, list(range(NCORES)), trace=trace)
    last_result = res

    outs = []
    for c in range(NCORES):
        a = res.results[c]["out"]                    # [T, 128, KD*BL]
        a = a.reshape(t_steps, P, KD, BL).transpose(3, 0, 2, 1).reshape(BL, t_steps, D)
        outs.append(a)
    return np.ascontiguousarray(np.concatenate(outs, axis=0)).astype(np.float32)

